# revision 1
# baseline (speedup 1.0000x reference)
"""Correlation-volume kernel for Trainium2 (8 NeuronCores, SPMD).

Problem: inputs (B=4, N=2, C=128, H=128, W=128) fp32.
  q = floor(inputs * 1e10) / 1e10  (straight-through quantization, fp32)
  src = q[:, 0], tgt = q[:, 1]
  out[b, dy*21+dx, h, w] = mean_c src[b,c,h,w] * tgt[b,c,h+dy-10,w+dx-10]
  (zero padding outside), out shape (4, 441, 128, 128) fp32.

Strategy:
  - Shard batch(4) x H-half(2) across 8 cores, data parallel, no collectives.
  - Host precomputes q, pre-blocks src into 128-pixel stationary tiles
    (16 h x 8 w), zero-pads tgt; one packed fp32 input per core.
  - Device: per block, 2 matmuls (K=C=128, M=128 pixels,
    N=18 tgt rows x 28 tgt cols = 504) -> PSUM; DVE/ACT copies -> SBUF;
    dense dump of the blocked Gram rectangles to DRAM (2.3x output volume).
  - Input load + float32r rounding is chunked so early blocks overlap the
    remaining input DMA (Tile deps are range-granular).
  - Host extracts the valid (dy, dx) band with a single strided view
    (the "skew" is unexpressible by on-chip engines; numpy does it free).

Knobs:
  MM_DT:   "float32r" (default, rel err ~1.5e-4, 1 cyc/row)
           "float32"  (rel err ~1.2e-7, 4 cyc/row -> PE-bound)
           "bfloat16" (rel err ~2.4e-3)
  DUMP_DT: "float32" (default) | "bfloat16" (halves dump DMA, adds ~2e-3 err)
"""

import sys

if "/opt/trn_rl_repo" not in sys.path:
    sys.path.insert(0, "/opt/trn_rl_repo")

import numpy as np

B, NIN, C, H, W = 4, 2, 128, 128, 128
KH = KW = 21
QS = np.float32(1e10)
HHALF = 64            # rows per core
HB, WB = 16, 8        # pixel block on stationary (M = 128)
NHB, NWB = HHALF // HB, W // WB      # 4, 16
RN2 = 18              # target rows per matmul (2 matmuls -> 36 = HB + 20)
WN = WB + 20          # 28 target cols per block
TROWS, TCOLS = HHALF + 20, W + 20    # 84, 148 padded target per core
SRC_F = HHALF * W                    # 8192
TGT_F = TROWS * TCOLS                # 12432
PACK_F = SRC_F + TGT_F
NBLK = NHB * NWB                     # 64
GRP = 2                              # blocks per dump DMA
NGRP = NBLK // GRP                   # 32
MM_DT = "float32r"
DUMP_DT = "float32"

_nc_cache = None


def _build_nc():
    from contextlib import ExitStack

    from concourse import bacc, mybir, tile
    from concourse._compat import with_exitstack

    nc = bacc.Bacc("TRN2")
    pack = nc.declare_dram_parameter(
        "pack", [C, PACK_F], mybir.dt.float32, isOutput=False
    )
    dt_dump = getattr(mybir.dt, DUMP_DT)
    out = nc.declare_dram_parameter(
        "out", [NGRP, 2, 128, GRP * 504], dt_dump, isOutput=True
    )
    dt_mm = getattr(mybir.dt, MM_DT)

    # input chunks: 4 tgt row-bands (21 rows each) + 4 src hb-bands
    tgt_chunks = [
        (SRC_F + t0 * TCOLS, SRC_F + t1 * TCOLS)
        for t0, t1 in ((0, 21), (21, 42), (42, 63), (63, TROWS))
    ]
    src_chunks = [
        (hb * NWB * 128, (hb + 1) * NWB * 128) for hb in range(NHB)
    ]
    # emission order: enough for hb=0 first, then the rest
    chunk_order = [
        tgt_chunks[0], tgt_chunks[1], src_chunks[0],
        tgt_chunks[2], src_chunks[1],
        tgt_chunks[3], src_chunks[2], src_chunks[3],
    ]

    @with_exitstack
    def kern(ctx: ExitStack, tc: tile.TileContext):
        nc = tc.nc
        sbp = ctx.enter_context(tc.tile_pool(name="inp", bufs=1))
        psa = ctx.enter_context(tc.tile_pool(name="psa", bufs=2, space="PSUM"))
        psb = ctx.enter_context(tc.tile_pool(name="psb", bufs=2, space="PSUM"))
        sta = ctx.enter_context(tc.tile_pool(name="sta", bufs=2))
        stb = ctx.enter_context(tc.tile_pool(name="stb", bufs=2))

        if dt_mm == mybir.dt.float32:
            pk = sbp.tile([C, PACK_F], mybir.dt.float32, tag="pk")
            for lo, hi in chunk_order:
                nc.sync.dma_start(pk[:, lo:hi], pack[:, lo:hi])
            data = pk
        else:
            pk = sbp.tile([C, PACK_F], mybir.dt.float32, tag="pk")
            pkr = sbp.tile([C, PACK_F], dt_mm, tag="pkr")
            for lo, hi in chunk_order:
                nc.sync.dma_start(pk[:, lo:hi], pack[:, lo:hi])
                nc.vector.tensor_copy(pkr[:, lo:hi], pk[:, lo:hi])
            data = pkr

        src2 = data[:, 0:SRC_F]
        tgt3 = data[:, SRC_F:].rearrange("c (t v) -> c t v", t=TROWS)

        for g in range(NGRP):
            sA = sta.tile([128, GRP * 504], dt_dump)
            sB = stb.tile([128, GRP * 504], dt_dump)
            for k in range(GRP):
                blk = g * GRP + k
                hb, wb = divmod(blk, NWB)
                t0, w0 = hb * HB, wb * WB
                lhs = src2[:, blk * 128 : (blk + 1) * 128]
                pA = psa.tile([128, 504], mybir.dt.float32)
                pB = psb.tile([128, 504], mybir.dt.float32)
                nc.tensor.matmul(
                    pA[:], lhs, tgt3[:, t0 : t0 + RN2, w0 : w0 + WN],
                    start=True, stop=True,
                )
                nc.tensor.matmul(
                    pB[:], lhs, tgt3[:, t0 + RN2 : t0 + 2 * RN2, w0 : w0 + WN],
                    start=True, stop=True,
                )
                nc.vector.tensor_copy(sA[:, k * 504 : (k + 1) * 504], pA[:])
                nc.scalar.copy(sB[:, k * 504 : (k + 1) * 504], pB[:])
            nc.sync.dma_start(out[g, 0], sA[:])
            nc.sync.dma_start(out[g, 1], sB[:])

    with tile.TileContext(nc) as tc:
        kern(tc)
    nc.finalize()
    return nc


def _get_nc():
    global _nc_cache
    if _nc_cache is None:
        _nc_cache = _build_nc()
    return _nc_cache


def _pack_inputs(q: np.ndarray) -> list[dict]:
    """Per-core packed input: blocked src + zero-padded tgt."""
    in_maps = []
    for core in range(8):
        b, half = core // 2, core % 2
        h0 = half * HHALF
        src = q[b, 0, :, h0 : h0 + HHALF, :]            # (C, 64, 128)
        srcb = (
            src.reshape(C, NHB, HB, NWB, WB)
            .transpose(0, 1, 3, 2, 4)                   # (C, hb, wb, h_l, w_l)
            .reshape(C, SRC_F)
        )
        tgt = np.zeros((C, TROWS, TCOLS), np.float32)
        lo, hi = h0 - 10, h0 + HHALF + 10
        clo, chi = max(lo, 0), min(hi, H)
        tgt[:, clo - lo : chi - lo, 10 : 10 + W] = q[b, 1, :, clo:chi, :]
        pack = np.concatenate([srcb, tgt.reshape(C, TGT_F)], axis=1)
        in_maps.append({"pack": np.ascontiguousarray(pack)})
    return in_maps


def _unscramble(results: list[dict]) -> np.ndarray:
    """Extract the valid (dy, dx) band from each core's dense Gram dump."""
    out = np.empty((B, KH * KW, H, W), np.float32)
    for core in range(8):
        b, half = core // 2, core % 2
        h0 = half * HHALF
        arr = np.asarray(results[core]["out"])
        if arr.dtype != np.float32:
            arr = arr.astype(np.float32)
        # (NGRP, 2, 128, GRP*504) -> [blk, p, t, v] with t = j*18 + r contiguous
        arr = arr.reshape(NGRP, 2, 128, GRP, RN2, WN)    # g j p k r v
        arr = arr.transpose(0, 3, 2, 1, 4, 5)            # g k p j r v
        arr = np.ascontiguousarray(arr).reshape(NHB, NWB, 128, 2 * RN2, WN)
        s_hb, s_wb, s_p, s_t, s_v = arr.strides
        V = np.lib.stride_tricks.as_strided(
            arr,
            shape=(NHB, NWB, HB, WB, KH, KW),
            strides=(s_hb, s_wb, WB * s_p + s_t, s_p + s_v, s_t, s_v),
        )
        # [hb, wb, h_l, w_l, dy, dx] -> [dy, dx, hb, h_l, wb, w_l]
        oc = V.transpose(4, 5, 0, 2, 1, 3).reshape(KH * KW, HHALF, W)
        out[b, :, h0 : h0 + HHALF, :] = oc
    out *= np.float32(1.0 / C)
    return out


def _run(inputs: np.ndarray, trace: bool = False, trace_kwargs: dict | None = None):
    from concourse.bass_utils import run_bass_kernel_spmd

    x = np.asarray(inputs, dtype=np.float32)
    assert x.shape == (B, NIN, C, H, W), x.shape
    q = np.floor(x * QS) / QS        # fp32 ops, matches the jax reference
    in_maps = _pack_inputs(q)
    nc = _get_nc()
    res = run_bass_kernel_spmd(
        nc, in_maps, core_ids=list(range(8)), trace=trace,
        **(trace_kwargs or {}),
    )
    out = _unscramble(res.results)
    return out, res


def kernel(inputs: np.ndarray) -> np.ndarray:
    out, _ = _run(inputs, trace=False)
    return out

